# revision 1
# baseline (speedup 1.0000x reference)
"""Trainium2 Bass kernel for nn_CausalAttention (GNN message passing).

Math (reference):
    pairs[e] = [img[:, src[e]] ; text[:, tgt[e]]]          # B == H == 128
    a[e]     = sigmoid(w2 . relu(W1 @ pairs[e] + b1) + b2) # per-edge gate
    att_img[b, i] = sum_{e: src[e]=i} a[e] * text[b, tgt[e]]
    att_txt[b, t] = sum_{e: tgt[e]=t} a[e] * img[b, src[e]]

Architecture: output-column sharding, fully on-chip (no dma_gather /
dma_scatter_add — the former is descriptor-generation-bound on the Q7,
the latter races on duplicate indices on HW).
Core c owns att_img[:, Wc] and att_txt[:, Wc], Wc = [128c, 128c+128).
For the img pipe (txt pipe symmetric, roles swapped):
  - edges with src in Wc, bucketed by w = tgt >> 7 (8 fixed-capacity
    buckets of 5 blocks of 128 edge slots; unused slots are dummies).
  - tables in SBUF: txtT8[lo, w, b] = text[b, 128w+lo],
    V8[lo, w, h] = (W1_txt @ text).T likewise, U_winT[loc, h] for Wc.
  - per bucket: transposed one-hot masks from host-replicated key rows
      ohKT[loc, e] = (srcloc[e] == loc),  ohLT[lo, e] = (tgtlo[e] == lo)
    h = relu(U_winT.T @ ohKT + V8[w].T @ ohLT + b1)   (PE matmuls)
    a = sigmoid(h.T @ w2 + b2)                        (per-block N=1 mm)
  - per block: M_w[lo, loc] += ohlo.T @ (a * ohK)     (PE, PSUM accum)
  - tail: att[:, loc] = sum_w txtT8[w].T @ M_w        (8 matmuls)
Host just concatenates the 8 column slices. Everything f32.
"""

import sys

for _p in ("/opt/trn_rl_repo", "/root/.axon_site/_ro/trn_rl_repo"):
    if _p not in sys.path:
        sys.path.insert(0, _p)

import numpy as np

import concourse.bass as bass
import concourse.tile as tile
from concourse import bacc, mybir

P = 128
DIM = 1024
E = 32768
NCORES = 8
NW = 8            # hi buckets
BPW = 5           # blocks per bucket (capacity 640 vs mean 512, +6 sigma)
NBLK = NW * BPW   # 40
EC = NBLK * P     # 5120 edge slots per pipeline
BW = BPW * P      # 640 edges per bucket

F32 = mybir.dt.float32
F16 = mybir.dt.float16
I8 = mybir.dt.int8

IS_EQ = mybir.AluOpType.is_equal
MULT = mybir.AluOpType.mult


def _build_program():
    nc = bacc.Bacc(None, target_bir_lowering=False, debug=False)

    img = nc.dram_tensor("img", [P, DIM], F32, kind="ExternalInput")
    txt = nc.dram_tensor("txt", [P, DIM], F32, kind="ExternalInput")
    img_t = nc.dram_tensor("img_t", [DIM, P], F32, kind="ExternalInput")
    txt_t = nc.dram_tensor("txt_t", [DIM, P], F32, kind="ExternalInput")
    win_d = nc.dram_tensor("win_d", [P, 2 * P], F32, kind="ExternalInput")
    w1t_img = nc.dram_tensor("w1t_img", [P, P], F32, kind="ExternalInput")
    w1t_txt = nc.dram_tensor("w1t_txt", [P, P], F32, kind="ExternalInput")
    cst_d = nc.dram_tensor("cst_d", [P, 3], F32, kind="ExternalInput")
    pin = {}
    for s in ("i", "t"):
        pin[s] = dict(
            ohkt=nc.dram_tensor(f"{s}_ohkt", [P, EC], F16, kind="ExternalInput"),
            ohlt=nc.dram_tensor(f"{s}_ohlt", [P, EC], F16, kind="ExternalInput"),
            ohlo=nc.dram_tensor(f"{s}_ohlo", [P, EC], F16, kind="ExternalInput"),
        )
    meta_d = nc.dram_tensor("meta_d", [P, 4 * NBLK], F32, kind="ExternalInput")
    out_img = nc.dram_tensor("out_img", [P, P], F32, kind="ExternalOutput")
    out_txt = nc.dram_tensor("out_txt", [P, P], F32, kind="ExternalOutput")

    with tile.TileContext(nc) as tc:
        with (
            tc.tile_pool(name="const", bufs=1) as cp,
            tc.tile_pool(name="work", bufs=3) as wp,
            tc.tile_pool(name="psH", bufs=2, space="PSUM") as psH,
            tc.tile_pool(name="psM", bufs=1, space="PSUM") as psM,
            tc.tile_pool(name="psS", bufs=1, space="PSUM") as psS,
        ):
            w1i_s = cp.tile([P, P], F32)
            w1x_s = cp.tile([P, P], F32)
            cst_s = cp.tile([P, 3], F32)
            iota_f = cp.tile([P, P], F32)
            iota16 = cp.tile([P, P], F16)
            iota_i = cp.tile([P, P], mybir.dt.int32)
            win_s = cp.tile([P, 2 * P], F32)
            img_s = cp.tile([P, DIM], F32)
            txt_s = cp.tile([P, DIM], F32)
            txtT8 = cp.tile([P, NW, P], F32)
            imgT8 = cp.tile([P, NW, P], F32)
            U8hi = cp.tile([P, NW, P], F16)
            U8lo = cp.tile([P, NW, P], F16)
            V8hi = cp.tile([P, NW, P], F16)
            V8lo = cp.tile([P, NW, P], F16)
            UwinThi = cp.tile([P, P], F16)
            UwinTlo = cp.tile([P, P], F16)
            VwinThi = cp.tile([P, P], F16)
            VwinTlo = cp.tile([P, P], F16)

            nc.sync.dma_start(w1i_s[:], w1t_img[:])
            nc.sync.dma_start(w1x_s[:], w1t_txt[:])
            nc.sync.dma_start(img_s[:], img[:])
            nc.sync.dma_start(txt_s[:], txt[:])
            nc.sync.dma_start(win_s[:], win_d[:])
            nc.sync.dma_start(cst_s[:], cst_d[:])
            b1_s = cst_s[:, 0:1]
            w2_s = cst_s[:, 1:2]
            b2_s = cst_s[:, 2:3]
            imgw_s = win_s[:, :P]
            txtw_s = win_s[:, P:]
            nc.sync.dma_start(
                txtT8[:], txt_t[:].rearrange("(w lo) b -> lo w b", lo=P)
            )
            nc.sync.dma_start(
                imgT8[:], img_t[:].rearrange("(w lo) b -> lo w b", lo=P)
            )
            nc.gpsimd.iota(iota_i[:], pattern=[[1, P]], base=0, channel_multiplier=0)
            nc.vector.tensor_copy(iota_f[:], iota_i[:])
            nc.vector.tensor_copy(iota16[:], iota_f[:])

            rep_s = {}
            meta_pack = cp.tile([P, 4 * NBLK], F32)
            nc.sync.dma_start(meta_pack[:], meta_d[:])
            meta_s = {
                ("i", "loc8"): meta_pack[:, 0 * NBLK : 1 * NBLK],
                ("i", "lo8"): meta_pack[:, 1 * NBLK : 2 * NBLK],
                ("t", "loc8"): meta_pack[:, 2 * NBLK : 3 * NBLK],
                ("t", "lo8"): meta_pack[:, 3 * NBLK : 4 * NBLK],
            }
            for s in ("i", "t"):
                for k in ("ohkt", "ohlt", "ohlo"):
                    t_ = cp.tile([P, EC], F16, tag=f"{s}{k}")
                    nc.scalar.dma_start(t_[:], pin[s][k][:])
                    rep_s[(s, k)] = t_


            # U8[lo, w, h] = (W1_img @ img).T rows (fp16 hi + residual);
            # UwinT likewise for Wc
            def build_pair(dst_hi, dst_lo, lhsT, rhs):
                ps = psH.tile([P, BW], F32, tag="h_ps", name=f"bp{id(dst_hi)}")
                nc.tensor.matmul(ps[:, :P], lhsT, rhs, start=True, stop=True)
                nc.scalar.copy(dst_hi, ps[:, :P])
                nc.vector.tensor_tensor(
                    out=dst_lo, in0=ps[:, :P], in1=dst_hi,
                    op=mybir.AluOpType.subtract,
                )

            for w in range(NW):
                build_pair(U8hi[:, w, :], U8lo[:, w, :],
                           img_s[:, w * P : (w + 1) * P], w1i_s[:])
                build_pair(V8hi[:, w, :], V8lo[:, w, :],
                           txt_s[:, w * P : (w + 1) * P], w1x_s[:])
            build_pair(UwinThi[:], UwinTlo[:], imgw_s, w1i_s[:])
            build_pair(VwinThi[:], VwinTlo[:], txtw_s, w1x_s[:])

            for side, arbT8, arbWhi, arbWlo, winThi, winTlo, out_d in (
                ("i", txtT8, V8hi, V8lo, UwinThi, UwinTlo, out_img),
                ("t", imgT8, U8hi, U8lo, VwinThi, VwinTlo, out_txt),
            ):
                ohkt_s = rep_s[(side, "ohkt")]
                ohlt_s = rep_s[(side, "ohlt")]
                ohlo_s = rep_s[(side, "ohlo")]
                loc8 = meta_s[(side, "loc8")]
                lo8 = meta_s[(side, "lo8")]
                m_ps0 = psM.tile([P, 4 * P], F32, tag="m0")
                m_ps1 = psM.tile([P, 4 * P], F32, tag="m1")
                m_ps = [m_ps0, m_ps1]
                acc = psS.tile([P, P], F32, tag="acc")
                a_ps = psS.tile([P, NBLK], F32, tag="a_ps")

                # ---- phase A: per-edge gate a ----
                for w in range(NW):
                    e0 = w * BW
                    ohKT = ohkt_s[:, e0 : e0 + BW]
                    ohLT = ohlt_s[:, e0 : e0 + BW]
                    h_ps = psH.tile([P, BW], F32, tag="h_ps")
                    for mi, (st, oh_) in enumerate((
                        (winThi[:], ohKT), (winTlo[:], ohKT),
                        (arbWhi[:, w, :], ohLT), (arbWlo[:, w, :], ohLT),
                    )):
                        st_ap = st[:] if hasattr(st, "tile_id") else st
                        for o, n in ((0, 512), (512, P)):
                            nc.tensor.matmul(
                                h_ps[:, o : o + n], st_ap, oh_[:, o : o + n],
                                start=(mi == 0), stop=(mi == 3),
                            )
                    h_s = wp.tile([P, BW], F32, tag="h_s")
                    nc.scalar.activation(
                        h_s[:], h_ps[:], mybir.ActivationFunctionType.Relu,
                        bias=b1_s,
                    )
                    for j in range(BPW):
                        b = w * BPW + j
                        nc.tensor.matmul(
                            a_ps[:, b : b + 1], h_s[:, j * P : (j + 1) * P],
                            w2_s, start=True, stop=True,
                        )
                a_s = wp.tile([P, NBLK], F32, tag="a_s")
                nc.scalar.activation(
                    a_s[:], a_ps[:], mybir.ActivationFunctionType.Sigmoid,
                    bias=b2_s,
                )
                a_hi16 = wp.tile([P, NBLK], F16, tag="a_hi16")
                a_hif = wp.tile([P, NBLK], F32, tag="a_hif")
                a_lof = wp.tile([P, NBLK], F32, tag="a_lof")
                nc.vector.tensor_copy(a_hi16[:], a_s[:])
                nc.vector.tensor_copy(a_hif[:], a_hi16[:])
                nc.vector.tensor_tensor(
                    out=a_lof[:], in0=a_s[:], in1=a_hif[:],
                    op=mybir.AluOpType.subtract,
                )

                # ---- phase B: M_w[lo, loc] += ohlo.T @ (a * ohK) ----
                for b in range(NBLK):
                    w, j = b // BPW, b % BPW
                    ohlo = ohlo_s[:, b * P : (b + 1) * P]
                    ohKh = wp.tile([P, P], F16, tag="ohKh")
                    ohKl = wp.tile([P, P], F16, tag="ohKl")
                    nc.vector.tensor_scalar(
                        out=ohKh[:], in0=iota16[:],
                        scalar1=loc8[:, b : b + 1], scalar2=a_hif[:, b : b + 1],
                        op0=IS_EQ, op1=MULT,
                    )
                    nc.vector.tensor_scalar(
                        out=ohKl[:], in0=iota16[:],
                        scalar1=loc8[:, b : b + 1], scalar2=a_lof[:, b : b + 1],
                        op0=IS_EQ, op1=MULT,
                    )
                    mslice = m_ps[w // 4][:, (w % 4) * P : (w % 4 + 1) * P]
                    nc.tensor.matmul(
                        mslice, ohlo, ohKh[:],
                        start=(j == 0), stop=False, skip_group_check=True,
                    )
                    nc.tensor.matmul(
                        mslice, ohlo, ohKl[:],
                        start=False, stop=(j == BPW - 1), skip_group_check=True,
                    )

                # ---- tail: att[:, loc] = sum_w arbT8[w].T @ M_w ----
                for w in range(NW):
                    m_s = wp.tile([P, P], F32, tag="m_s")
                    nc.vector.tensor_copy(
                        m_s[:], m_ps[w // 4][:, (w % 4) * P : (w % 4 + 1) * P]
                    )
                    nc.tensor.matmul(
                        acc[:], arbT8[:, w, :], m_s[:],
                        start=(w == 0), stop=(w == NW - 1),
                        skip_group_check=True,
                    )
                out_sb = wp.tile([P, P], F32, tag="out_sb")
                nc.vector.tensor_copy(out_sb[:], acc[:])
                nc.sync.dma_start(out_d[:], out_sb[:])

    nc.compile()
    return nc


_PROGRAM = None


def _get_program():
    global _PROGRAM
    if _PROGRAM is None:
        _PROGRAM = _build_program()
    return _PROGRAM


def _pipe_arrays(key, arb, base):
    """key: bucketing key values (src for img pipe); arb: the other endpoint.
    Returns repk, repl [P, EC] i8 row-replicated, loc8/lo8 [P, NBLK] f32."""
    kloc = key - base                 # 0..127
    w = arb >> 7                      # bucket
    lo = arb & 127
    slots = np.full(EC, -1, np.int64)  # slot -> edge index or -1
    fill = np.zeros(NW, np.int64)
    order = np.argsort(w, kind="stable")
    for ei in order:
        wb = w[ei]
        assert fill[wb] < BW, f"bucket overflow: {fill[wb]}"
        slots[wb * BW + fill[wb]] = ei
        fill[wb] += 1
    klocs = np.full(EC, -1, np.int64)
    los = np.full(EC, -1, np.int64)
    used = slots >= 0
    klocs[used] = kloc[slots[used]]
    los[used] = lo[slots[used]]
    rng = np.arange(P)
    ohkt = np.ascontiguousarray((klocs[None, :] == rng[:, None]).astype(np.float16))
    ohlt = np.ascontiguousarray((los[None, :] == rng[:, None]).astype(np.float16))
    # ohlo[e % P, b*P + lo] = (los[e] == lo), block-diagonal [e, lo] tiles
    ohlo = np.zeros((P, EC), np.float16)
    for b in range(NBLK):
        blk = los[b * P : (b + 1) * P]
        ohlo[:, b * P : (b + 1) * P] = blk[:, None] == rng[None, :]
    ohlo = np.ascontiguousarray(ohlo)
    # col layout [P, NBLK]: edge slot e at [e % 128, e // 128]
    loc8 = np.ascontiguousarray(klocs.astype(np.float32).reshape(NBLK, P).T)
    lo8 = np.ascontiguousarray(los.astype(np.float32).reshape(NBLK, P).T)
    return ohkt, ohlt, ohlo, loc8, lo8


def _make_in_maps(img_features, text_features, src, tgt, W1, b1, w2, b2):
    img = np.ascontiguousarray(img_features.astype(np.float32))
    txt = np.ascontiguousarray(text_features.astype(np.float32))
    imgT = np.ascontiguousarray(img.T)
    txtT = np.ascontiguousarray(txt.T)
    w1t_img = np.ascontiguousarray(W1[:, :P].T.astype(np.float32))
    w1t_txt = np.ascontiguousarray(W1[:, P:].T.astype(np.float32))
    b1c = np.ascontiguousarray(b1.astype(np.float32).reshape(P, 1))
    w2c = np.ascontiguousarray(w2.astype(np.float32).reshape(P, 1))
    b2c = np.full((P, 1), np.float32(b2), dtype=np.float32)
    src = np.asarray(src).astype(np.int64)
    tgt = np.asarray(tgt).astype(np.int64)

    in_maps = []
    for c in range(NCORES):
        base = c * P
        m = {
            "img": img, "txt": txt, "img_t": imgT, "txt_t": txtT,
            "win_d": np.ascontiguousarray(np.concatenate(
                [img[:, base : base + P], txt[:, base : base + P]], axis=1)),
            "w1t_img": w1t_img, "w1t_txt": w1t_txt,
            "cst_d": np.ascontiguousarray(
                np.concatenate([b1c, w2c, b2c], axis=1)),
        }
        metas = {}
        for s, key, arb in (("i", src, tgt), ("t", tgt, src)):
            sel = (key >= base) & (key < base + P)
            ohkt, ohlt, ohlo, loc8, lo8 = _pipe_arrays(key[sel], arb[sel], base)
            m[f"{s}_ohkt"] = ohkt
            m[f"{s}_ohlt"] = ohlt
            m[f"{s}_ohlo"] = ohlo
            metas[s] = (loc8, lo8)
        m["meta_d"] = np.ascontiguousarray(np.concatenate(
            [metas["i"][0], metas["i"][1], metas["t"][0], metas["t"][1]],
            axis=1))
        in_maps.append(m)
    return in_maps


def _run(inputs, trace=False):
    from concourse.bass_utils import run_bass_kernel_spmd

    nc = _get_program()
    in_maps = _make_in_maps(**inputs)
    res = run_bass_kernel_spmd(
        nc, in_maps, core_ids=list(range(NCORES)), trace=trace
    )
    att_img = np.concatenate([r["out_img"] for r in res.results], axis=1)
    att_txt = np.concatenate([r["out_txt"] for r in res.results], axis=1)
    return (np.ascontiguousarray(att_img), np.ascontiguousarray(att_txt)), res


def kernel(**inputs):
    out, _ = _run(inputs, trace=False)
    return out



# revision 5
# speedup vs baseline: 1.0690x; 1.0690x over previous
"""Trainium2 Bass kernel for nn_CausalAttention (GNN message passing).

Math (reference):
    pairs[e] = [img[:, src[e]] ; text[:, tgt[e]]]          # B == H == 128
    a[e]     = sigmoid(w2 . relu(W1 @ pairs[e] + b1) + b2) # per-edge gate
    att_img[b, i] = sum_{e: src[e]=i} a[e] * text[b, tgt[e]]
    att_txt[b, t] = sum_{e: tgt[e]=t} a[e] * img[b, src[e]]

v2 architecture: deduplicated edges + host reduction.
Core c owns the edges with src in Wc = [128c, 128c+128). It computes
  - att_img[:, Wc] exactly (all edges for those columns live here), and
  - a PARTIAL att_txt[:, :] (the contribution of its edges); the host
    sums the 8 partials. No collectives needed.
Everything runs in fp16 (tolerance is 2e-2; fp16 lands ~1e-3):
  - U = W1_img @ img[:, Wc] and V = W1_txt @ text built on-chip (9 mm)
  - phase A per tgt-bucket w: h = relu(U.T @ ohKT + V8[w].T @ ohLT + b1)
    via host-shipped one-hot tables (key-major layout [key, edge]);
    za[e] = h_blk.T @ w2 (N=1 matmuls), a = sigmoid(za + b2)
  - phase B per 128-edge block: on-chip one-hots ohka[e, loc] =
    a*(kloc==loc) (DVE) and ohlo[e, lo] = (lo_e==lo) (gpsimd);
    M_T[lo, loc] += ohlo.T @ ohka   (A-block, transposed layout)
    M_N[loc, lo] += ohka.T @ ohlo   (A-block, natural layout)
  - tails per bucket: acc_img += ttT8[w].T @ M_T[w];
    part[:, w*128:...] = imgwinT.T @ M_N[w]
PSUM budget (8 banks): h x2 (4) + mtn (1) + acc (1) + part (2).
"""

import sys

for _p in ("/opt/trn_rl_repo", "/root/.axon_site/_ro/trn_rl_repo"):
    if _p not in sys.path:
        sys.path.insert(0, _p)

import numpy as np

import concourse.bass as bass
import concourse.tile as tile
from concourse import bacc, mybir

P = 128
DIM = 1024
NCORES = 8
NW = 8            # tgt-hi buckets

F32 = mybir.dt.float32
F16 = mybir.dt.float16

IS_EQ = mybir.AluOpType.is_equal
MULT = mybir.AluOpType.mult
RELU = mybir.ActivationFunctionType.Relu
SIGMOID = mybir.ActivationFunctionType.Sigmoid


def _build_program(bpw):
    nblk = NW * bpw       # blocks total
    bw = bpw * P          # edge slots per bucket
    ec = nblk * P         # edge slots total

    nc = bacc.Bacc(None, target_bir_lowering=False, debug=False)

    txt16_d = nc.dram_tensor("txt16", [P, DIM], F16, kind="ExternalInput")
    ttT8_d = nc.dram_tensor("ttT8", [P, NW * P], F16, kind="ExternalInput")
    iw_d = nc.dram_tensor("iw16", [P, P], F16, kind="ExternalInput")
    iwT_d = nc.dram_tensor("iwT16", [P, P], F16, kind="ExternalInput")
    w1i_d = nc.dram_tensor("w1i16", [P, P], F16, kind="ExternalInput")
    w1x_d = nc.dram_tensor("w1x16", [P, P], F16, kind="ExternalInput")
    cst_d = nc.dram_tensor("cst", [P, 2], F32, kind="ExternalInput")
    w2h_d = nc.dram_tensor("w2h", [P, 1], F16, kind="ExternalInput")
    ohkt_d = nc.dram_tensor("ohkt", [P, ec], F16, kind="ExternalInput")
    ohlt_d = nc.dram_tensor("ohlt", [P, ec], F16, kind="ExternalInput")
    meta_d = nc.dram_tensor("meta", [P, 2 * nblk], F32, kind="ExternalInput")
    out_img = nc.dram_tensor("out_img", [P, P], F32, kind="ExternalOutput")
    out_part = nc.dram_tensor("out_part", [P, DIM], F32, kind="ExternalOutput")

    with tile.TileContext(nc) as tc:
        with (
            tc.tile_pool(name="const", bufs=1) as cp,
            tc.tile_pool(name="work", bufs=3) as wp,
            tc.tile_pool(name="psh", bufs=2, space="PSUM") as psh,
            tc.tile_pool(name="psm", bufs=1, space="PSUM") as psm,
            tc.tile_pool(name="pso", bufs=1, space="PSUM") as pso,
        ):
            txt16 = cp.tile([P, DIM], F16)
            ttT8 = cp.tile([P, NW, P], F16)
            iw_s = cp.tile([P, P], F16)
            iwT_s = cp.tile([P, P], F16)
            w1i_s = cp.tile([P, P], F16)
            w1x_s = cp.tile([P, P], F16)
            cst_s = cp.tile([P, 2], F32)
            w2h_s = cp.tile([P, 1], F16)
            ohkt_s = cp.tile([P, ec], F16)
            ohlt_s = cp.tile([P, ec], F16)
            meta_s = cp.tile([P, 2 * nblk], F32)
            iota16 = cp.tile([P, P], F16)
            V8 = cp.tile([P, NW, P], F16)
            UwinT = cp.tile([P, P], F16)
            m16T = cp.tile([P, NW, P], F16)
            m16N = cp.tile([P, NW, P], F16)
            a_s = cp.tile([P, nblk], F32)

            # small loads on the scalar queue, big tables chunked on sync
            nc.scalar.dma_start(cst_s[:], cst_d[:])
            nc.scalar.dma_start(w2h_s[:], w2h_d[:])
            nc.scalar.dma_start(meta_s[:], meta_d[:])
            nc.scalar.dma_start(w1i_s[:], w1i_d[:])
            nc.scalar.dma_start(w1x_s[:], w1x_d[:])
            nc.scalar.dma_start(iw_s[:], iw_d[:])
            nc.scalar.dma_start(iwT_s[:], iwT_d[:])
            nc.scalar.dma_start(txt16[:], txt16_d[:])
            nc.scalar.dma_start(
                ttT8[:], ttT8_d[:].rearrange("p (w b) -> p w b", w=NW)
            )
            for w in range(NW):
                nc.sync.dma_start(
                    ohkt_s[:, w * bw : (w + 1) * bw],
                    ohkt_d[:, w * bw : (w + 1) * bw],
                )
                nc.sync.dma_start(
                    ohlt_s[:, w * bw : (w + 1) * bw],
                    ohlt_d[:, w * bw : (w + 1) * bw],
                )
            b1c = cst_s[:, 0:1]
            b2c = cst_s[:, 1:2]
            loc8 = meta_s[:, 0:nblk]
            lo8 = meta_s[:, nblk : 2 * nblk]

            nc.gpsimd.iota(
                iota16[:], pattern=[[1, P]], base=0, channel_multiplier=0,
                allow_small_or_imprecise_dtypes=True,
            )

            # on-chip feature transforms: UwinT[loc,h], V8[lo,w,h] (fp16)
            HW = 640  # h psum tile free size (5*128); za tail at cols 640:648
            for k, (lhs, rhs, dst) in enumerate(
                [(iw_s[:], w1i_s[:], UwinT[:])]
                + [
                    (txt16[:, w * P : (w + 1) * P], w1x_s[:], V8[:, w, :])
                    for w in range(NW)
                ]
            ):
                bp = psh.tile([P, HW + 8], F32, tag="h", name=f"bld{k}")
                nc.tensor.matmul(bp[:, 0:P], lhs, rhs, start=True, stop=True)
                nc.vector.tensor_copy(dst, bp[:, 0:P])

            for w in range(NW):
                e0 = w * bw
                # ---- phase A: h = relu(U-term + V-term + b1) ----
                h_ps = psh.tile([P, HW + 8], F32, tag="h")
                for o, n in ((0, 512), (512, bw - 512)):
                    nc.tensor.matmul(
                        h_ps[:, o : o + n], UwinT[:],
                        ohkt_s[:, e0 + o : e0 + o + n],
                        start=True, stop=False,
                    )
                    nc.tensor.matmul(
                        h_ps[:, o : o + n], V8[:, w, :],
                        ohlt_s[:, e0 + o : e0 + o + n],
                        start=False, stop=True,
                    )
                h16 = wp.tile([P, bw], F16, tag="h16")
                nc.scalar.activation(h16[:], h_ps[:, 0:bw], RELU, bias=b1c)
                # ---- za[e] = h_blk.T @ w2; a = sigmoid(za + b2) ----
                for j in range(bpw):
                    nc.tensor.matmul(
                        h_ps[:, HW + j : HW + j + 1],
                        h16[:, j * P : (j + 1) * P], w2h_s[:],
                        start=True, stop=True, skip_group_check=True,
                    )
                nc.scalar.activation(
                    a_s[:, w * bpw : (w + 1) * bpw],
                    h_ps[:, HW : HW + bpw], SIGMOID, bias=b2c,
                )
                # ---- phase B: A-block outer products (both layouts).
                # One PSUM accumulation group may be open per 2KB zero
                # region at a time, so run the full mT group, then mN. ----
                mtn = psm.tile([P, 2 * P], F32, tag="mtn")
                ohkaB = wp.tile([P, bw], F16, tag="ohka")
                ohloB = wp.tile([P, bw], F16, tag="ohlo")
                for j in range(bpw):
                    b = w * bpw + j
                    nc.vector.tensor_scalar(
                        out=ohkaB[:, j * P : (j + 1) * P], in0=iota16[:],
                        scalar1=loc8[:, b : b + 1], scalar2=a_s[:, b : b + 1],
                        op0=IS_EQ, op1=MULT,
                    )
                    nc.gpsimd.tensor_scalar(
                        out=ohloB[:, j * P : (j + 1) * P], in0=iota16[:],
                        scalar1=lo8[:, b : b + 1], scalar2=None,
                        op0=IS_EQ,
                    )
                for j in range(bpw):
                    nc.tensor.matmul(
                        mtn[:, 0:P], ohloB[:, j * P : (j + 1) * P],
                        ohkaB[:, j * P : (j + 1) * P],
                        start=(j == 0), stop=(j == bpw - 1),
                        skip_group_check=True,
                    )
                for j in range(bpw):
                    nc.tensor.matmul(
                        mtn[:, P : 2 * P], ohkaB[:, j * P : (j + 1) * P],
                        ohloB[:, j * P : (j + 1) * P],
                        start=(j == 0), stop=(j == bpw - 1),
                        skip_group_check=True,
                    )
                nc.vector.tensor_copy(m16T[:, w, :], mtn[:, 0:P])
                nc.scalar.copy(m16N[:, w, :], mtn[:, P : 2 * P])

            # ---- tails ----
            acc = pso.tile([P, P], F32, tag="acc")
            part = pso.tile([P, DIM], F32, tag="part")
            for w in range(NW):
                nc.tensor.matmul(
                    acc[:], ttT8[:, w, :], m16T[:, w, :],
                    start=(w == 0), stop=(w == NW - 1), skip_group_check=True,
                )
                nc.tensor.matmul(
                    part[:, w * P : (w + 1) * P], iwT_s[:], m16N[:, w, :],
                    start=True, stop=True, skip_group_check=True,
                )
            out_sb = wp.tile([P, P], F32, tag="out_sb")
            nc.scalar.copy(out_sb[:], acc[:])
            nc.sync.dma_start(out_img[:], out_sb[:])
            part_sb = wp.tile([P, DIM], F32, tag="part_sb")
            nc.vector.tensor_copy(part_sb[:], part[:])
            nc.sync.dma_start(out_part[:], part_sb[:])

    nc.compile()
    return nc


_PROGRAMS = {}


def _get_program(bpw):
    if bpw not in _PROGRAMS:
        _PROGRAMS[bpw] = _build_program(bpw)
    return _PROGRAMS[bpw]


def _core_arrays(kloc, arb, bpw):
    """kloc: src-base (0..127) for this core's edges; arb: tgt values.
    Returns ohkt, ohlt [P, ec] f16 (key-major), loc8/lo8 [P, nblk] f32."""
    nblk = NW * bpw
    bw = bpw * P
    ec = nblk * P
    w = arb >> 7
    lo = arb & 127
    klocs = np.full(ec, -1, np.int64)
    los = np.full(ec, -1, np.int64)
    fill = np.zeros(NW, np.int64)
    order = np.argsort(w, kind="stable")
    for ei in order:
        wb = w[ei]
        s = wb * bw + fill[wb]
        klocs[s] = kloc[ei]
        los[s] = lo[ei]
        fill[wb] += 1
    rng = np.arange(P)
    ohkt = np.ascontiguousarray((klocs[None, :] == rng[:, None]).astype(np.float16))
    ohlt = np.ascontiguousarray((los[None, :] == rng[:, None]).astype(np.float16))
    # col layout [P, nblk]: edge slot s at [s % 128, s // 128]
    loc8 = np.ascontiguousarray(klocs.astype(np.float32).reshape(nblk, P).T)
    lo8 = np.ascontiguousarray(los.astype(np.float32).reshape(nblk, P).T)
    return ohkt, ohlt, loc8, lo8


def _make_in_maps(img_features, text_features, src, tgt, W1, b1, w2, b2, bpw):
    img = np.asarray(img_features, dtype=np.float32)
    txt = np.asarray(text_features, dtype=np.float32)
    src = np.asarray(src).astype(np.int64)
    tgt = np.asarray(tgt).astype(np.int64)
    txt16 = np.ascontiguousarray(txt.astype(np.float16))
    txtT = txt.T.astype(np.float16)                     # [1024, 128]
    ttT8 = np.ascontiguousarray(
        txtT.reshape(NW, P, P).transpose(1, 0, 2).reshape(P, NW * P)
    )                                                   # [lo, w*128+b]
    w1i16 = np.ascontiguousarray(W1[:, :P].T.astype(np.float16))
    w1x16 = np.ascontiguousarray(W1[:, P:].T.astype(np.float16))
    cst = np.ascontiguousarray(
        np.stack(
            [np.asarray(b1, np.float32),
             np.full(P, np.float32(b2), np.float32)], axis=1)
    )
    w2h = np.ascontiguousarray(np.asarray(w2, np.float16).reshape(P, 1))

    in_maps = []
    for c in range(NCORES):
        base = c * P
        sel = (src >= base) & (src < base + P)
        ohkt, ohlt, loc8, lo8 = _core_arrays(src[sel] - base, tgt[sel], bpw)
        iw = img[:, base : base + P].astype(np.float16)
        m = {
            "txt16": txt16, "ttT8": ttT8,
            "iw16": np.ascontiguousarray(iw),
            "iwT16": np.ascontiguousarray(iw.T),
            "w1i16": w1i16, "w1x16": w1x16,
            "cst": cst, "w2h": w2h,
            "ohkt": ohkt, "ohlt": ohlt,
            "meta": np.ascontiguousarray(np.concatenate([loc8, lo8], axis=1)),
        }
        in_maps.append(m)
    return in_maps


def _pick_bpw(src, tgt):
    src = np.asarray(src).astype(np.int64)
    tgt = np.asarray(tgt).astype(np.int64)
    mx = 0
    for c in range(NCORES):
        sel = (src >> 7) == c
        w = tgt[sel] >> 7
        mx = max(mx, int(np.bincount(w, minlength=NW).max()))
    return (mx + P - 1) // P


def _run(inputs, trace=False):
    from concourse.bass_utils import run_bass_kernel_spmd

    bpw = _pick_bpw(inputs["src"], inputs["tgt"])
    nc = _get_program(bpw)
    in_maps = _make_in_maps(**inputs, bpw=bpw)
    res = run_bass_kernel_spmd(
        nc, in_maps, core_ids=list(range(NCORES)), trace=trace
    )
    att_img = np.concatenate([r["out_img"] for r in res.results], axis=1)
    att_txt = np.sum([r["out_part"] for r in res.results], axis=0)
    return (
        np.ascontiguousarray(att_img.astype(np.float32)),
        np.ascontiguousarray(att_txt.astype(np.float32)),
    ), res


def kernel(**inputs):
    out, _ = _run(inputs, trace=False)
    return out


# revision 13
# speedup vs baseline: 2.0718x; 1.9380x over previous
"""Trainium2 Bass kernel for nn_CausalAttention (GNN message passing).

Math (reference):
    pairs[e] = [img[:, src[e]] ; text[:, tgt[e]]]          # B == H == 128
    a[e]     = sigmoid(w2 . relu(W1 @ pairs[e] + b1) + b2) # per-edge gate
    att_img[b, i] = sum_{e: src[e]=i} a[e] * text[b, tgt[e]]
    att_txt[b, t] = sum_{e: tgt[e]=t} a[e] * img[b, src[e]]

v2 architecture: deduplicated edges + host reduction.
Core c owns the edges with src in Wc = [128c, 128c+128). It computes
  - att_img[:, Wc] exactly (all edges for those columns live here), and
  - a PARTIAL att_txt[:, :] (the contribution of its edges); the host
    sums the 8 partials. No collectives needed.
Everything runs in fp16 (tolerance is 2e-2; fp16 lands ~1e-3):
  - U = W1_img @ img[:, Wc] and V = W1_txt @ text built on-chip (9 mm)
  - phase A per tgt-bucket w: h = relu(U.T @ ohKT + V8[w].T @ ohLT + b1)
    via host-shipped one-hot tables (key-major layout [key, edge]);
    za[e] = h_blk.T @ w2 (N=1 matmuls), a = sigmoid(za + b2)
  - phase B per 128-edge block: on-chip one-hots ohka[e, loc] =
    a*(kloc==loc) (DVE) and ohlo[e, lo] = (lo_e==lo) (gpsimd);
    M_T[lo, loc] += ohlo.T @ ohka   (A-block, transposed layout)
    M_N[loc, lo] += ohka.T @ ohlo   (A-block, natural layout)
  - tails per bucket: acc_img += ttT8[w].T @ M_T[w];
    part[:, w*128:...] = imgwinT.T @ M_N[w]
PSUM budget (8 banks): h x2 (4) + mtn (1) + acc (1) + part (2).
"""

import sys

for _p in ("/opt/trn_rl_repo", "/root/.axon_site/_ro/trn_rl_repo"):
    if _p not in sys.path:
        sys.path.insert(0, _p)

import numpy as np

import concourse.bass as bass
import concourse.tile as tile
from concourse import bacc, mybir

P = 128
DIM = 1024
NCORES = 8
NW = 8            # tgt-hi buckets

F32 = mybir.dt.float32
F16 = mybir.dt.float16

IS_EQ = mybir.AluOpType.is_equal
MULT = mybir.AluOpType.mult
RELU = mybir.ActivationFunctionType.Relu
SIGMOID = mybir.ActivationFunctionType.Sigmoid


def _build_program(bpw):
    nblk = NW * bpw       # blocks total
    bw = bpw * P          # edge slots per bucket
    ec = nblk * P         # edge slots total

    nc = bacc.Bacc(None, target_bir_lowering=False, debug=False)

    txt16_d = nc.dram_tensor("txt16", [P, DIM], F16, kind="ExternalInput")
    ttT8_d = nc.dram_tensor("ttT8", [P, NW * P], F16, kind="ExternalInput")
    iw_d = nc.dram_tensor("iw16", [P, P], F16, kind="ExternalInput")
    iwT_d = nc.dram_tensor("iwT16", [P, P], F16, kind="ExternalInput")
    w1i_d = nc.dram_tensor("w1i16", [P, P], F16, kind="ExternalInput")
    w1x_d = nc.dram_tensor("w1x16", [P, P], F16, kind="ExternalInput")
    cst_d = nc.dram_tensor("cst", [P, 2], F32, kind="ExternalInput")
    w2h_d = nc.dram_tensor("w2h", [P, 1], F16, kind="ExternalInput")
    ohkt_d = nc.dram_tensor("ohkt", [P, ec], F16, kind="ExternalInput")
    ohlt_d = nc.dram_tensor("ohlt", [P, ec], F16, kind="ExternalInput")
    ohlo_d = nc.dram_tensor("ohlo", [P, ec], F16, kind="ExternalInput")
    meta_d = nc.dram_tensor("meta", [P, 2 * nblk], F32, kind="ExternalInput")
    out_img = nc.dram_tensor("out_img", [P, P], F32, kind="ExternalOutput")
    out_part = nc.dram_tensor("out_part", [P, DIM], F32, kind="ExternalOutput")

    with tile.TileContext(nc) as tc:
        with (
            tc.tile_pool(name="const", bufs=1) as cp,
            tc.tile_pool(name="work", bufs=3) as wp,
            tc.tile_pool(name="psh", bufs=2, space="PSUM") as psh,
            tc.tile_pool(name="psm", bufs=1, space="PSUM") as psm,
            tc.tile_pool(name="pso", bufs=1, space="PSUM") as pso,
        ):
            txt16 = cp.tile([P, DIM], F16)
            ttT8 = cp.tile([P, NW, P], F16)
            iw_s = cp.tile([P, P], F16)
            iwT_s = cp.tile([P, P], F16)
            w1i_s = cp.tile([P, P], F16)
            w1x_s = cp.tile([P, P], F16)
            cst_s = cp.tile([P, 2], F32)
            w2h_s = cp.tile([P, 1], F16)
            ohkt_s = cp.tile([P, ec], F16)
            ohlt_s = cp.tile([P, ec], F16)
            ohlo_s = cp.tile([P, ec], F16)
            meta_s = cp.tile([P, 2 * nblk], F32)
            iota16 = cp.tile([P, P], F16)
            V8 = cp.tile([P, NW, P], F16)
            UwinT = cp.tile([P, P], F16)
            m16T = cp.tile([P, NW, P], F16)
            m16N = cp.tile([P, NW, P], F16)
            a_s = cp.tile([P, nblk], F32)

            # small loads on the scalar queue, big tables chunked on sync
            nc.scalar.dma_start(cst_s[:], cst_d[:])
            nc.scalar.dma_start(w2h_s[:], w2h_d[:])
            nc.scalar.dma_start(meta_s[:], meta_d[:])
            nc.scalar.dma_start(w1i_s[:], w1i_d[:])
            nc.scalar.dma_start(w1x_s[:], w1x_d[:])
            nc.scalar.dma_start(iw_s[:], iw_d[:])
            nc.scalar.dma_start(iwT_s[:], iwT_d[:])
            nc.scalar.dma_start(txt16[:], txt16_d[:])
            nc.scalar.dma_start(
                ttT8[:], ttT8_d[:].rearrange("p (w b) -> p w b", w=NW)
            )
            for w in range(NW):
                nc.sync.dma_start(
                    ohkt_s[:, w * bw : (w + 1) * bw],
                    ohkt_d[:, w * bw : (w + 1) * bw],
                )
                nc.sync.dma_start(
                    ohlt_s[:, w * bw : (w + 1) * bw],
                    ohlt_d[:, w * bw : (w + 1) * bw],
                )
                nc.scalar.dma_start(
                    ohlo_s[:, w * bw : (w + 1) * bw],
                    ohlo_d[:, w * bw : (w + 1) * bw],
                )
            b1c = cst_s[:, 0:1]
            b2c = cst_s[:, 1:2]
            loc8 = meta_s[:, 0:nblk]
            lo8 = meta_s[:, nblk : 2 * nblk]

            nc.gpsimd.iota(
                iota16[:], pattern=[[1, P]], base=0, channel_multiplier=0,
                allow_small_or_imprecise_dtypes=True,
            )

            # on-chip feature transforms: UwinT[loc,h], V8[lo,w,h] (fp16)
            HW = 640  # h psum tile free size (5*128); za tail at cols 640:648
            for k, (lhs, rhs, dst) in enumerate(
                [(iw_s[:], w1i_s[:], UwinT[:])]
                + [
                    (txt16[:, w * P : (w + 1) * P], w1x_s[:], V8[:, w, :])
                    for w in range(NW)
                ]
            ):
                bp = psh.tile([P, HW + 8], F32, tag="h", name=f"bld{k}")
                nc.tensor.matmul(bp[:, 0:P], lhs, rhs, start=True, stop=True)
                nc.scalar.copy(dst, bp[:, 0:P])

            for w in range(NW):
                e0 = w * bw
                # ---- phase A: h = relu(U-term + V-term + b1) ----
                h_ps = psh.tile([P, HW + 8], F32, tag="h")
                for o, n in ((0, 512), (512, bw - 512)):
                    nc.tensor.matmul(
                        h_ps[:, o : o + n], UwinT[:],
                        ohkt_s[:, e0 + o : e0 + o + n],
                        start=True, stop=False,
                    )
                    nc.tensor.matmul(
                        h_ps[:, o : o + n], V8[:, w, :],
                        ohlt_s[:, e0 + o : e0 + o + n],
                        start=False, stop=True,
                    )
                h16 = wp.tile([P, bw], F16, tag="h16")
                nc.scalar.activation(h16[:], h_ps[:, 0:bw], RELU, bias=b1c)
                # ---- za[e] = h_blk.T @ w2; a = sigmoid(za + b2) ----
                for j in range(bpw):
                    nc.tensor.matmul(
                        h_ps[:, HW + j : HW + j + 1],
                        h16[:, j * P : (j + 1) * P], w2h_s[:],
                        start=True, stop=True, skip_group_check=True,
                    )
                nc.scalar.activation(
                    a_s[:, w * bpw : (w + 1) * bpw],
                    h_ps[:, HW : HW + bpw], SIGMOID, bias=b2c,
                )
                # ---- phase B: A-block outer products (both layouts).
                # One PSUM accumulation group may be open per 2KB zero
                # region at a time, so run the full mT group, then mN. ----
                mtn = psm.tile([P, 2 * P], F32, tag="mtn")
                ohkaB = wp.tile([P, bw], F16, tag="ohka")
                for j in range(bpw):
                    b = w * bpw + j
                    nc.vector.tensor_scalar(
                        out=ohkaB[:, j * P : (j + 1) * P], in0=iota16[:],
                        scalar1=loc8[:, b : b + 1], scalar2=a_s[:, b : b + 1],
                        op0=IS_EQ, op1=MULT,
                    )
                for j in range(bpw):
                    nc.tensor.matmul(
                        mtn[:, 0:P], ohlo_s[:, (w * bpw + j) * P : (w * bpw + j + 1) * P],
                        ohkaB[:, j * P : (j + 1) * P],
                        start=(j == 0), stop=(j == bpw - 1),
                        skip_group_check=True,
                    )
                for j in range(bpw):
                    nc.tensor.matmul(
                        mtn[:, P : 2 * P], ohkaB[:, j * P : (j + 1) * P],
                        ohlo_s[:, (w * bpw + j) * P : (w * bpw + j + 1) * P],
                        start=(j == 0), stop=(j == bpw - 1),
                        skip_group_check=True,
                    )
                nc.vector.tensor_copy(m16T[:, w, :], mtn[:, 0:P])
                nc.scalar.copy(m16N[:, w, :], mtn[:, P : 2 * P])

            # ---- tails ----
            acc = pso.tile([P, P], F32, tag="acc")
            part = pso.tile([P, DIM], F32, tag="part")
            for w in range(NW):
                nc.tensor.matmul(
                    acc[:], ttT8[:, w, :], m16T[:, w, :],
                    start=(w == 0), stop=(w == NW - 1), skip_group_check=True,
                )
                nc.tensor.matmul(
                    part[:, w * P : (w + 1) * P], iwT_s[:], m16N[:, w, :],
                    start=True, stop=True, skip_group_check=True,
                )
            out_sb = wp.tile([P, P], F32, tag="out_sb")
            nc.scalar.copy(out_sb[:], acc[:])
            nc.sync.dma_start(out_img[:], out_sb[:])
            part_sb = wp.tile([P, DIM], F32, tag="part_sb")
            nc.vector.tensor_copy(part_sb[:], part[:])
            nc.sync.dma_start(out_part[:], part_sb[:])

    nc.compile()
    return nc


_PROGRAMS = {}


def _get_program(bpw):
    if bpw not in _PROGRAMS:
        _PROGRAMS[bpw] = _build_program(bpw)
    return _PROGRAMS[bpw]


def _core_arrays(kloc, arb, bpw):
    """kloc: src-base (0..127) for this core's edges; arb: tgt values.
    Returns ohkt, ohlt [P, ec] f16 (key-major), loc8/lo8 [P, nblk] f32."""
    nblk = NW * bpw
    bw = bpw * P
    ec = nblk * P
    w = arb >> 7
    lo = arb & 127
    klocs = np.full(ec, -1, np.int64)
    los = np.full(ec, -1, np.int64)
    fill = np.zeros(NW, np.int64)
    order = np.argsort(w, kind="stable")
    for ei in order:
        wb = w[ei]
        s = wb * bw + fill[wb]
        klocs[s] = kloc[ei]
        los[s] = lo[ei]
        fill[wb] += 1
    rng = np.arange(P)
    ohkt = np.ascontiguousarray((klocs[None, :] == rng[:, None]).astype(np.float16))
    ohlt = np.ascontiguousarray((los[None, :] == rng[:, None]).astype(np.float16))
    # ohlo[e % P, b*P + lo] = (los[e] == lo): per-block [e, lo] tiles
    ohlo = np.zeros((P, ec), np.float16)
    losb = los.reshape(nblk, P)
    for b in range(nblk):
        ohlo[:, b * P : (b + 1) * P] = losb[b][:, None] == rng[None, :]
    ohlo = np.ascontiguousarray(ohlo)
    # col layout [P, nblk]: edge slot s at [s % 128, s // 128]
    loc8 = np.ascontiguousarray(klocs.astype(np.float32).reshape(nblk, P).T)
    lo8 = np.ascontiguousarray(los.astype(np.float32).reshape(nblk, P).T)
    return ohkt, ohlt, ohlo, loc8, lo8


def _make_in_maps(img_features, text_features, src, tgt, W1, b1, w2, b2, bpw):
    img = np.asarray(img_features, dtype=np.float32)
    txt = np.asarray(text_features, dtype=np.float32)
    src = np.asarray(src).astype(np.int64)
    tgt = np.asarray(tgt).astype(np.int64)
    txt16 = np.ascontiguousarray(txt.astype(np.float16))
    txtT = txt.T.astype(np.float16)                     # [1024, 128]
    ttT8 = np.ascontiguousarray(
        txtT.reshape(NW, P, P).transpose(1, 0, 2).reshape(P, NW * P)
    )                                                   # [lo, w*128+b]
    w1i16 = np.ascontiguousarray(W1[:, :P].T.astype(np.float16))
    w1x16 = np.ascontiguousarray(W1[:, P:].T.astype(np.float16))
    cst = np.ascontiguousarray(
        np.stack(
            [np.asarray(b1, np.float32),
             np.full(P, np.float32(b2), np.float32)], axis=1)
    )
    w2h = np.ascontiguousarray(np.asarray(w2, np.float16).reshape(P, 1))

    in_maps = []
    for c in range(NCORES):
        base = c * P
        sel = (src >= base) & (src < base + P)
        ohkt, ohlt, ohlo, loc8, lo8 = _core_arrays(src[sel] - base, tgt[sel], bpw)
        iw = img[:, base : base + P].astype(np.float16)
        m = {
            "txt16": txt16, "ttT8": ttT8,
            "iw16": np.ascontiguousarray(iw),
            "iwT16": np.ascontiguousarray(iw.T),
            "w1i16": w1i16, "w1x16": w1x16,
            "cst": cst, "w2h": w2h,
            "ohkt": ohkt, "ohlt": ohlt, "ohlo": ohlo,
            "meta": np.ascontiguousarray(np.concatenate([loc8, lo8], axis=1)),
        }
        in_maps.append(m)
    return in_maps


def _pick_bpw(src, tgt):
    src = np.asarray(src).astype(np.int64)
    tgt = np.asarray(tgt).astype(np.int64)
    mx = 0
    for c in range(NCORES):
        sel = (src >> 7) == c
        w = tgt[sel] >> 7
        mx = max(mx, int(np.bincount(w, minlength=NW).max()))
    return (mx + P - 1) // P


def _run(inputs, trace=False):
    from concourse.bass_utils import run_bass_kernel_spmd

    bpw = _pick_bpw(inputs["src"], inputs["tgt"])
    nc = _get_program(bpw)
    in_maps = _make_in_maps(**inputs, bpw=bpw)
    res = run_bass_kernel_spmd(
        nc, in_maps, core_ids=list(range(NCORES)), trace=trace
    )
    att_img = np.concatenate([r["out_img"] for r in res.results], axis=1)
    att_txt = np.sum([r["out_part"] for r in res.results], axis=0)
    return (
        np.ascontiguousarray(att_img.astype(np.float32)),
        np.ascontiguousarray(att_txt.astype(np.float32)),
    ), res


def kernel(**inputs):
    out, _ = _run(inputs, trace=False)
    return out
